# revision 1
# baseline (speedup 1.0000x reference)
"""Trainium2 Bass kernel for nn_Attention_5927054869144.

Channel-attention over [B=8, C=64, H=256, W=256] inputs. Data-parallel over
batch: one batch element per NeuronCore (8 cores), no collectives.

Per-core pipeline (x_b viewed as [64, 65536], spatial blocks of 8192):
  1. qkvT projection with x-chunk stationary on the PE -> q/k/v directly in
     spatial-partition layout (fp16 operands, fp32 PSUM).
  2. Per-head-pair dots matmuls from gathered (alpha, i) column APs,
     accumulated in PSUM over all spatial tiles (softmax scale folded into
     Wq/bq host-side).
  3. Unnormalized softmax: exp(x - max) on ScalarE with accumulated row sums;
     1/rowsum folded into per-head copies of Wo^T.
  4. M_h^T = expdots_h @ (Wo^T * recip) via tiny matmuls.
  5. v transposed to dim-partition layout via TensorE gather-transposes
     (fp16 PSUM), then final output = M_h^T.T @ v_dp, evacuated fp32 and
     DMAed per 2048-column chunk.
"""

import os
import sys

import numpy as np

for _p in ("/opt/trn_rl_repo", "/root/.axon_site/_ro/trn_rl_repo"):
    if os.path.isdir(_p) and _p not in sys.path:
        sys.path.insert(0, _p)

from concourse import bacc, mybir, tile  # noqa: E402
from concourse.bass_utils import run_bass_kernel_spmd  # noqa: E402

F32 = mybir.dt.float32
F16 = mybir.dt.float16

HEADS = 8
C = 64
HW = 65536          # 256*256 spatial positions per batch element
BL = HW // HEADS    # 8192, per-head block length
NQ = 4              # spatial quarters (within-block n ranges)
QL = BL // NQ       # 2048 within-block positions per quarter
TPQ = QL // 128     # 16 tile groups per quarter
N_GROUPS = BL // 128  # 64 total tile groups
CHUNK_B = 4096      # phase-B output chunk columns per head

LAST_RESULTS = None


def _build_kernel(hw=HW):
    bl = hw // HEADS
    ql = bl // NQ
    tpq = ql // 128
    n_groups = bl // 128
    chunk_b = min(CHUNK_B, bl)
    s5n = chunk_b // 512
    c0n = bl // chunk_b

    nc = bacc.Bacc("TRN2", target_bir_lowering=False, debug=False)
    x_d = nc.dram_tensor("x", [65, hw], F32, kind="ExternalInput")
    wqkv_d = nc.dram_tensor("wqkv", [65, 192], F16, kind="ExternalInput")
    wot_d = nc.dram_tensor("wot", [128, 64], F32, kind="ExternalInput")
    ident_d = nc.dram_tensor("ident", [128, 128], F16, kind="ExternalInput")
    out_d = nc.dram_tensor("out", [64, hw], F32, kind="ExternalOutput")

    x_ap = x_d.ap()
    out_ap = out_d.ap()
    # [pr, s, o, n] view of the output: head h = 2*pr + s
    out_v = out_ap.rearrange("o (p s n) -> p s o n", p=4, s=2)

    with tile.TileContext(nc) as tc:
        with (
            tc.tile_pool(name="consts", bufs=1) as cpool,
            tc.tile_pool(name="pers", bufs=1) as pers,
            tc.tile_pool(name="dotsp", bufs=1, space="PSUM") as dotspool,
        ):
            wqkv_sb = cpool.tile([65, 192], F16)
            wot_sb = cpool.tile([128, 64], F32)
            ident_sb = cpool.tile([128, 128], F16)
            nc.sync.dma_start(out=wqkv_sb[:, :], in_=wqkv_d.ap()[:, :])
            nc.sync.dma_start(out=wot_sb[:, :], in_=wot_d.ap()[:, :])
            nc.sync.dma_start(out=ident_sb[:, :], in_=ident_d.ap()[:, :])

            # v in dim-partition layout: [pair, d(0:64 even head / 64:128 odd), n]
            vdp = pers.tile([128, 4 * bl], F16)
            dots_ps = [
                dotspool.tile([128, 128], F32, name=f"dots{p}") for p in range(4)
            ]

            # ---------------- Phase A ----------------
            vdp_v = vdp.rearrange("p (r n) -> p r n", r=4)
            with (
                tc.tile_pool(name="xq", bufs=2) as xpool,
                tc.tile_pool(name="slots", bufs=5) as slotpool,
                tc.tile_pool(name="projp", bufs=2, space="PSUM") as projpool,
                tc.tile_pool(name="vtrp", bufs=2, space="PSUM") as vtrpool,
            ):
                x_blk = x_ap.rearrange("p (i n) -> p i n", i=8)
                slots = {}

                def consume(g):
                    # dots + v-transpose for a group whose slot is fully evac'd
                    slot = slots.pop(g)
                    vt = vtrpool.tile([128, 512], F16, name="vt")
                    for pr in range(4):
                        qs = slot[:, 128 * pr: 128 * pr + 128]
                        ks = slot[:, 512 + 128 * pr: 512 + 128 * pr + 128]
                        vs = slot[:, 1024 + 128 * pr: 1024 + 128 * pr + 128]
                        nc.tensor.matmul(
                            dots_ps[pr][:, :],
                            lhsT=qs,
                            rhs=ks,
                            start=(g == 0),
                            stop=(g == n_groups - 1),
                        )
                        nc.tensor.transpose(
                            vt[:, pr * 128:(pr + 1) * 128], vs, ident_sb[:, :]
                        )
                    voff = g * 128
                    vdst = vdp_v[:, :, voff:voff + 128]
                    if g % 2 == 0:
                        nc.vector.tensor_copy(vdst, vt[:, :])
                    else:
                        nc.scalar.copy(vdst, vt[:, :])

                for q in range(NQ):
                    xq = xpool.tile([65, 8 * ql], F16, name="xq")
                    nc.gpsimd.dma_start(
                        out=xq.rearrange("p (i n) -> p i n", i=8),
                        in_=x_blk[:, :, q * ql:(q + 1) * ql],
                    )
                    for t0 in range(tpq):
                        g = q * tpq + t0
                        # slot cols: r*512 + head*64 + i*8 + alpha (alpha contiguous)
                        slot = slotpool.tile([128, 1536], F16, name="slot")
                        slot_sc = slot.rearrange(
                            "p (r h i a) -> p i r h a", r=3, h=8, i=8, a=8
                        )
                        slots[g] = slot
                        for ip in range(4):  # chunk pairs (2i, 2i+1)
                            pp = projpool.tile([128, 384], F32, name="pp")
                            for c in range(2):
                                i = 2 * ip + c
                                nc.tensor.matmul(
                                    pp[:, c * 192:(c + 1) * 192],
                                    lhsT=xq[:, i * ql + t0 * 128:
                                            i * ql + t0 * 128 + 128],
                                    rhs=wqkv_sb[:, :],
                                    start=True,
                                    stop=True,
                                )
                            dst = slot_sc[:, 2 * ip: 2 * ip + 2, :, :, :]
                            if ip % 2 == 0:
                                nc.vector.tensor_copy(dst, pp[:, :])
                            else:
                                nc.scalar.copy(dst, pp[:, :])
                        if g >= 3:
                            consume(g - 3)
                for g in (n_groups - 3, n_groups - 2, n_groups - 1):
                    consume(g)

            # ---------------- Softmax + output ----------------
            with (
                tc.tile_pool(name="smx", bufs=1) as smx,
                tc.tile_pool(name="mhp", bufs=1, space="PSUM") as mhpool,
                tc.tile_pool(name="finp", bufs=3, space="PSUM") as finpool,
                tc.tile_pool(name="outs", bufs=3) as outpool,
            ):
                negmax = smx.tile([128, 4], F32)
                rowsum = smx.tile([128, 4], F32)
                recip = smx.tile([128, 4], F32)
                exps = smx.tile([128, 4 * 64], F16)
                wots = smx.tile([128, 4 * 64], F16)
                mh_sb = smx.tile([128, 4 * 64], F16)
                mh_ps = mhpool.tile([128, 64], F32)

                for h in range(HEADS):
                    b = (h % 2) * 64
                    pr = h // 2
                    dsl = dots_ps[pr][b:b + 64, b:b + 64]
                    nc.vector.reduce_max(
                        negmax[b:b + 64, pr:pr + 1], dsl,
                        axis=mybir.AxisListType.X, negate=True,
                    )
                    nc.scalar.activation(
                        exps[b:b + 64, pr * 64:(pr + 1) * 64], dsl,
                        mybir.ActivationFunctionType.Exp,
                        bias=negmax[b:b + 64, pr:pr + 1],
                        scale=1.0,
                        accum_out=rowsum[b:b + 64, pr:pr + 1],
                    )
                    nc.vector.reciprocal(
                        recip[b:b + 64, pr:pr + 1], rowsum[b:b + 64, pr:pr + 1]
                    )
                    nc.vector.tensor_scalar_mul(
                        wots[b:b + 64, pr * 64:(pr + 1) * 64],
                        wot_sb[b:b + 64, :],
                        recip[b:b + 64, pr:pr + 1],
                    )
                    nc.tensor.matmul(
                        mh_ps[b:b + 64, :],
                        lhsT=exps[b:b + 64, pr * 64:(pr + 1) * 64],
                        rhs=wots[b:b + 64, pr * 64:(pr + 1) * 64],
                        start=True,
                        stop=True,
                    )
                    nc.vector.tensor_copy(
                        mh_sb[b:b + 64, pr * 64:(pr + 1) * 64], mh_ps[b:b + 64, :]
                    )

                dma_engs = [nc.sync, nc.scalar, nc.gpsimd]
                ci = 0
                for pr in range(4):
                    for c0 in range(c0n):
                        outsb = outpool.tile([128, chunk_b], F32, name="outsb")
                        for s5 in range(s5n):
                            fp_ = finpool.tile([128, 512], F32, name="fp_")
                            n0 = pr * bl + c0 * chunk_b + s5 * 512
                            nc.tensor.matmul(
                                fp_[0:64, :],
                                lhsT=mh_sb[0:64, pr * 64:(pr + 1) * 64],
                                rhs=vdp[0:64, n0:n0 + 512],
                                start=True,
                                stop=True,
                            )
                            nc.tensor.matmul(
                                fp_[64:128, :],
                                lhsT=mh_sb[64:128, pr * 64:(pr + 1) * 64],
                                rhs=vdp[64:128, n0:n0 + 512],
                                start=True,
                                stop=True,
                            )
                            if s5 % 2 == 0:
                                nc.vector.tensor_copy(
                                    outsb[:, s5 * 512:(s5 + 1) * 512], fp_[:, :])
                            else:
                                nc.scalar.copy(
                                    outsb[:, s5 * 512:(s5 + 1) * 512], fp_[:, :])
                        dma_engs[ci % 3].dma_start(
                            out=out_v[pr, :, :, c0 * chunk_b:(c0 + 1) * chunk_b],
                            in_=outsb[:, :],
                        )
                        ci += 1

    nc.compile()
    return nc


_NC_CACHE = {}


def _get_nc(hw=HW):
    if hw not in _NC_CACHE:
        _NC_CACHE[hw] = _build_kernel(hw)
    return _NC_CACHE[hw]


def _host_inputs(Wq, bq, Wk, bk, Wv, bv, Wo):
    scale = 64 ** -0.5
    wqkv = np.zeros((65, 192), np.float16)
    wqkv[:64, 0:64] = (Wq.T * scale).astype(np.float16)
    wqkv[64, 0:64] = (bq * scale).astype(np.float16)
    wqkv[:64, 64:128] = Wk.T.astype(np.float16)
    wqkv[64, 64:128] = bk.astype(np.float16)
    wqkv[:64, 128:192] = Wv.T.astype(np.float16)
    wqkv[64, 128:192] = bv.astype(np.float16)
    # kernel uses c' = i*8 + alpha ordering; original c = alpha*8 + i
    pi = np.array([(c % 8) * 8 + c // 8 for c in range(64)])
    wotp = Wo.T[pi]
    wot = np.concatenate([wotp, wotp], axis=0).astype(np.float32)
    ident = np.eye(128, dtype=np.float16)
    return wqkv, wot, ident


def kernel(x, Wq, bq, Wk, bk, Wv, bv, Wo):
    global LAST_RESULTS
    B = x.shape[0]
    hw = x.shape[2] * x.shape[3]
    nc = _get_nc(hw)
    wqkv, wot, ident = _host_inputs(Wq, bq, Wk, bk, Wv, bv, Wo)

    in_maps = []
    for bidx in range(B):
        x65 = np.empty((65, hw), np.float32)
        x65[:64] = x[bidx].reshape(64, hw)
        x65[64] = 1.0
        in_maps.append({"x": x65, "wqkv": wqkv, "wot": wot, "ident": ident})

    trace = bool(os.environ.get("KERNEL_TRACE"))
    res = run_bass_kernel_spmd(
        nc, in_maps, core_ids=list(range(B)), trace=trace
    )
    LAST_RESULTS = res
    out = np.stack(
        [res.results[bidx]["out"].reshape(64, HEADS, hw // HEADS)
         for bidx in range(B)]
    )
    return out



# revision 6
# speedup vs baseline: 1.4561x; 1.4561x over previous
"""Trainium2 Bass kernel for nn_Attention_5927054869144.

Channel-attention over [B=8, C=64, H=256, W=256] inputs. Data-parallel over
batch: one batch element per NeuronCore (8 cores), no collectives.

Per-core pipeline (x_b viewed as [64, 65536], spatial blocks of 8192):
  1. qkvT projection with x-chunk stationary on the PE -> q/k/v directly in
     spatial-partition layout (fp16 operands, fp32 PSUM).
  2. Per-head-pair dots matmuls from gathered (alpha, i) column APs,
     accumulated in PSUM over all spatial tiles (softmax scale folded into
     Wq/bq host-side).
  3. Unnormalized softmax: exp(x - max) on ScalarE with accumulated row sums;
     1/rowsum folded into per-head copies of Wo^T.
  4. P_h^T = expdots_h @ (Wo^T * recip) written into a block-diagonal
     [128,128] lhsT per head pair.
  5. v transposed to dim-partition layout via TensorE gather-transposes
     (fp16 PSUM), then final output = blockdiag(P)^T @ v_dp in single K=128
     matmuls, evacuated fp16 and written back via engine-rotated SWDGE DMAs.

I/O is fp16 on the wire: x is cast + quarter-major reordered host-side (so
input DMA descriptors are 32KB/partition), output is written fp16 and
upcast host-side.
"""

import os
import sys

import numpy as np

for _p in ("/opt/trn_rl_repo", "/root/.axon_site/_ro/trn_rl_repo"):
    if os.path.isdir(_p) and _p not in sys.path:
        sys.path.insert(0, _p)

from concourse import bacc, mybir, tile  # noqa: E402
from concourse.bass_utils import run_bass_kernel_spmd  # noqa: E402

F32 = mybir.dt.float32
F16 = mybir.dt.float16

HEADS = 8
C = 64
HW = 65536          # 256*256 spatial positions per batch element
BL = HW // HEADS    # 8192, per-head block length
NQ = 4              # spatial quarters (within-block n ranges)
QL = BL // NQ       # 2048 within-block positions per quarter
TPQ = QL // 128     # 16 tile groups per quarter
N_GROUPS = BL // 128  # 64 total tile groups

LAST_RESULTS = None


def _build_kernel(hw=HW):
    bl = hw // HEADS
    ql = bl // NQ
    tpq = ql // 128
    n_groups = bl // 128

    nc = bacc.Bacc("TRN2", target_bir_lowering=False, debug=False)
    # x columns are quarter-major host-side: (q, i, ql)
    x_d = nc.dram_tensor("x", [65, hw], F16, kind="ExternalInput")
    wqkv_d = nc.dram_tensor("wqkv", [65, 192], F16, kind="ExternalInput")
    wot_d = nc.dram_tensor("wot", [128, 64], F32, kind="ExternalInput")
    ident_d = nc.dram_tensor("ident", [128, 128], F16, kind="ExternalInput")
    out_d = nc.dram_tensor("out", [64, hw], F16, kind="ExternalOutput")

    x_ap = x_d.ap()
    out_ap = out_d.ap()
    # [pr, s, o, n] view of the output: head h = 2*pr + s
    out_v = out_ap.rearrange("o (p s n) -> p s o n", p=4, s=2)

    with tile.TileContext(nc) as tc:
        with (
            tc.tile_pool(name="consts", bufs=1) as cpool,
            tc.tile_pool(name="pers", bufs=1) as pers,
            tc.tile_pool(name="smx", bufs=1) as smx,
            tc.tile_pool(name="dotsp", bufs=1, space="PSUM") as dotspool,
        ):
            wqkv_sb = cpool.tile([65, 192], F16)
            wot_sb = cpool.tile([128, 64], F32)
            ident_sb = cpool.tile([128, 128], F16)
            nc.sync.dma_start(out=wqkv_sb[:, :], in_=wqkv_d.ap()[:, :])
            nc.sync.dma_start(out=wot_sb[:, :], in_=wot_d.ap()[:, :])
            nc.sync.dma_start(out=ident_sb[:, :], in_=ident_d.ap()[:, :])

            # v in dim-partition layout: [pair, d(0:64 even head / 64:128 odd), n]
            vdp = pers.tile([128, 4 * bl], F16)
            dots_ps = [
                dotspool.tile([128, 128], F32, name=f"dots{p}") for p in range(4)
            ]

            # block-diagonal P^T per head pair; off-diag blocks stay zero
            mhbd = [smx.tile([128, 128], F16, name=f"mhbd{p}") for p in range(4)]
            for p in range(4):
                nc.vector.memset(mhbd[p][:, :], 0.0)

            # ---------------- Phase A ----------------
            vdp_v = vdp.rearrange("p (r n) -> p r n", r=4)
            with (
                tc.tile_pool(name="xq", bufs=2) as xpool,
                tc.tile_pool(name="slots", bufs=5) as slotpool,
                tc.tile_pool(name="projp", bufs=2, space="PSUM") as projpool,
                tc.tile_pool(name="vtrp", bufs=2, space="PSUM") as vtrpool,
            ):
                slots = {}

                def consume(g):
                    # dots + v-transpose for a group whose slot is fully evac'd
                    slot = slots.pop(g)
                    vt = vtrpool.tile([128, 512], F16, name="vt")
                    for pr in range(4):
                        qs = slot[:, 128 * pr: 128 * pr + 128]
                        ks = slot[:, 512 + 128 * pr: 512 + 128 * pr + 128]
                        vs = slot[:, 1024 + 128 * pr: 1024 + 128 * pr + 128]
                        nc.tensor.matmul(
                            dots_ps[pr][:, :],
                            lhsT=qs,
                            rhs=ks,
                            start=(g == 0),
                            stop=(g == n_groups - 1),
                        )
                        nc.tensor.transpose(
                            vt[:, pr * 128:(pr + 1) * 128], vs, ident_sb[:, :]
                        )
                    voff = g * 128
                    vdst = vdp_v[:, :, voff:voff + 128]
                    if g % 2 == 0:
                        nc.vector.tensor_copy(vdst, vt[:, :])
                    else:
                        nc.scalar.copy(vdst, vt[:, :])

                for q in range(NQ):
                    xq = xpool.tile([65, 8 * ql], F16, name="xq")
                    # quarter-major DRAM layout: contiguous 32KB per partition
                    nc.gpsimd.dma_start(
                        out=xq[:, :],
                        in_=x_ap[:, q * 8 * ql:(q + 1) * 8 * ql],
                    )
                    for t0 in range(tpq):
                        g = q * tpq + t0
                        # slot cols: r*512 + head*64 + i*8 + alpha (alpha contiguous)
                        slot = slotpool.tile([128, 1536], F16, name="slot")
                        slot_sc = slot.rearrange(
                            "p (r h i a) -> p i r h a", r=3, h=8, i=8, a=8
                        )
                        slots[g] = slot
                        for ip in range(4):  # chunk pairs (2i, 2i+1)
                            pp = projpool.tile([128, 384], F32, name="pp")
                            for c in range(2):
                                i = 2 * ip + c
                                nc.tensor.matmul(
                                    pp[:, c * 192:(c + 1) * 192],
                                    lhsT=xq[:, i * ql + t0 * 128:
                                            i * ql + t0 * 128 + 128],
                                    rhs=wqkv_sb[:, :],
                                    start=True,
                                    stop=True,
                                )
                            dst = slot_sc[:, 2 * ip: 2 * ip + 2, :, :, :]
                            if ip % 2 == 0:
                                nc.vector.tensor_copy(dst, pp[:, :])
                            else:
                                nc.scalar.copy(dst, pp[:, :])
                        if g >= 3:
                            consume(g - 3)
                for g in (n_groups - 3, n_groups - 2, n_groups - 1):
                    consume(g)

            # ---------------- Softmax + output ----------------
            with (
                tc.tile_pool(name="mhp", bufs=1, space="PSUM") as mhpool,
                tc.tile_pool(name="finp", bufs=3, space="PSUM") as finpool,
                tc.tile_pool(name="outs", bufs=2) as outpool,
            ):
                for pr in range(4):
                    negmax = smx.tile([128, 1], F32, name=f"negmax{pr}")
                    rowsum = smx.tile([128, 1], F32, name=f"rowsum{pr}")
                    recip = smx.tile([128, 1], F32, name=f"recip{pr}")
                    exps = smx.tile([128, 64], F16, name=f"exps{pr}")
                    wots = smx.tile([128, 64], F16, name=f"wots{pr}")
                    mh_ps = mhpool.tile([128, 64], F32, name="mh_ps")
                    for s in range(2):  # head = 2*pr + s
                        b = s * 64
                        dsl = dots_ps[pr][b:b + 64, b:b + 64]
                        nc.vector.reduce_max(
                            negmax[b:b + 64, 0:1], dsl,
                            axis=mybir.AxisListType.X, negate=True,
                        )
                        nc.scalar.activation(
                            exps[b:b + 64, :], dsl,
                            mybir.ActivationFunctionType.Exp,
                            bias=negmax[b:b + 64, 0:1],
                            scale=1.0,
                            accum_out=rowsum[b:b + 64, 0:1],
                        )
                        nc.vector.reciprocal(
                            recip[b:b + 64, 0:1], rowsum[b:b + 64, 0:1]
                        )
                        nc.vector.tensor_scalar_mul(
                            wots[b:b + 64, :],
                            wot_sb[b:b + 64, :],
                            recip[b:b + 64, 0:1],
                        )
                        nc.tensor.matmul(
                            mh_ps[b:b + 64, :],
                            lhsT=exps[b:b + 64, :],
                            rhs=wots[b:b + 64, :],
                            start=True,
                            stop=True,
                        )
                        # diag block of the pair's block-diagonal P^T
                        nc.vector.tensor_copy(
                            mhbd[pr][b:b + 64, b:b + 64], mh_ps[b:b + 64, :]
                        )

                    outsb = outpool.tile([128, bl], F16, name="outsb")
                    for s5 in range(bl // 512):
                        fp_ = finpool.tile([128, 512], F32, name="fp_")
                        n0 = pr * bl + s5 * 512
                        nc.tensor.matmul(
                            fp_[:, :],
                            lhsT=mhbd[pr][:, :],
                            rhs=vdp[:, n0:n0 + 512],
                            start=True,
                            stop=True,
                        )
                        if s5 % 2 == 0:
                            nc.vector.tensor_copy(
                                outsb[:, s5 * 512:(s5 + 1) * 512], fp_[:, :])
                        else:
                            nc.scalar.copy(
                                outsb[:, s5 * 512:(s5 + 1) * 512], fp_[:, :])
                    # 4 SWDGE stores of 32 descriptors each: the packet
                    # rotation spreads consecutive calls across SDMA engines
                    for j in range(4):
                        s, o0 = j // 2, (j % 2) * 32
                        nc.gpsimd.dma_start(
                            out=out_v[pr, s, o0:o0 + 32, :],
                            in_=outsb[64 * s + o0: 64 * s + o0 + 32, :],
                        )

    nc.compile()
    return nc


_NC_CACHE = {}


def _get_nc(hw=HW):
    if hw not in _NC_CACHE:
        _NC_CACHE[hw] = _build_kernel(hw)
    return _NC_CACHE[hw]


def _host_inputs(Wq, bq, Wk, bk, Wv, bv, Wo):
    scale = 64 ** -0.5
    wqkv = np.zeros((65, 192), np.float16)
    wqkv[:64, 0:64] = (Wq.T * scale).astype(np.float16)
    wqkv[64, 0:64] = (bq * scale).astype(np.float16)
    wqkv[:64, 64:128] = Wk.T.astype(np.float16)
    wqkv[64, 64:128] = bk.astype(np.float16)
    wqkv[:64, 128:192] = Wv.T.astype(np.float16)
    wqkv[64, 128:192] = bv.astype(np.float16)
    # kernel uses c' = i*8 + alpha ordering; original c = alpha*8 + i
    pi = np.array([(c % 8) * 8 + c // 8 for c in range(64)])
    wotp = Wo.T[pi]
    wot = np.concatenate([wotp, wotp], axis=0).astype(np.float32)
    ident = np.eye(128, dtype=np.float16)
    return wqkv, wot, ident


def kernel(x, Wq, bq, Wk, bk, Wv, bv, Wo):
    global LAST_RESULTS
    B = x.shape[0]
    hw = x.shape[2] * x.shape[3]
    nc = _get_nc(hw)
    wqkv, wot, ident = _host_inputs(Wq, bq, Wk, bk, Wv, bv, Wo)

    ql = hw // HEADS // NQ
    in_maps = []
    for bidx in range(B):
        x65 = np.empty((65, hw), np.float16)
        x65[:64] = x[bidx].reshape(64, hw)
        x65[64] = 1.0
        # quarter-major column order: (q, i, ql) so each quarter load is
        # one contiguous 32KB descriptor per partition
        x65 = np.ascontiguousarray(
            x65.reshape(65, HEADS, NQ, ql).transpose(0, 2, 1, 3)
        ).reshape(65, hw)
        in_maps.append({"x": x65, "wqkv": wqkv, "wot": wot, "ident": ident})

    trace = bool(os.environ.get("KERNEL_TRACE"))
    res = run_bass_kernel_spmd(
        nc, in_maps, core_ids=list(range(B)), trace=trace
    )
    LAST_RESULTS = res
    out = np.stack(
        [res.results[bidx]["out"].reshape(64, HEADS, hw // HEADS)
         for bidx in range(B)]
    ).astype(np.float32)
    return out


# revision 10
# speedup vs baseline: 2.1205x; 1.4563x over previous
"""Trainium2 Bass kernel for nn_Attention_5927054869144.

Channel-attention over [B=8, C=64, H=256, W=256] inputs. Data-parallel over
batch: one batch element per NeuronCore (8 cores), no collectives.

Per-core pipeline (x_b viewed as [64, 65536], spatial blocks of 8192):
  1. qkvT projection with x-chunk stationary on the PE -> q/k/v directly in
     spatial-partition layout (fp16 operands, fp32 PSUM).
  2. Per-head-pair dots matmuls from gathered (alpha, i) column APs,
     accumulated in PSUM over all spatial tiles (softmax scale folded into
     Wq/bq host-side).
  3. Unnormalized softmax: exp(x - max) on ScalarE with accumulated row sums;
     1/rowsum folded into per-head copies of Wo^T.
  4. P_h^T = expdots_h @ (Wo^T * recip) written into a block-diagonal
     [128,128] lhsT per head pair.
  5. v transposed to dim-partition layout via TensorE gather-transposes
     (fp16 PSUM), then final output = blockdiag(P)^T @ v_dp in single K=128
     matmuls, evacuated fp16 and written back via engine-rotated SWDGE DMAs.

I/O is fp16 on the wire: x is cast + quarter-major reordered host-side (so
input DMA descriptors are 32KB/partition), output is written fp16 and
upcast host-side.
"""

import os
import sys

import numpy as np

for _p in ("/opt/trn_rl_repo", "/root/.axon_site/_ro/trn_rl_repo"):
    if os.path.isdir(_p) and _p not in sys.path:
        sys.path.insert(0, _p)

from concourse import bacc, mybir, tile  # noqa: E402
from concourse.bass_utils import run_bass_kernel_spmd  # noqa: E402

F32 = mybir.dt.float32
F16 = mybir.dt.float16

HEADS = 8
C = 64
HW = 65536          # 256*256 spatial positions per batch element
BL = HW // HEADS    # 8192, per-head block length
NQ = 4              # spatial quarters (within-block n ranges)
QL = BL // NQ       # 2048 within-block positions per quarter
TPQ = QL // 128     # 16 tile groups per quarter
N_GROUPS = BL // 128  # 64 total tile groups

LAST_RESULTS = None


def _build_kernel(hw=HW):
    bl = hw // HEADS
    ql = bl // NQ
    tpq = ql // 128
    n_groups = bl // 128

    nc = bacc.Bacc("TRN2", target_bir_lowering=False, debug=False)
    # x columns are quarter-major host-side: (q, i, ql)
    x_d = nc.dram_tensor("x", [65, hw], F16, kind="ExternalInput")
    wqkv_d = nc.dram_tensor("wqkv", [65, 192], F16, kind="ExternalInput")
    wot_d = nc.dram_tensor("wot", [128, 64], F32, kind="ExternalInput")
    ident_d = nc.dram_tensor("ident", [128, 128], F16, kind="ExternalInput")
    out_d = nc.dram_tensor("out", [64, hw], F16, kind="ExternalOutput")

    x_ap = x_d.ap()
    out_ap = out_d.ap()
    # [pr, s, o, n] view of the output: head h = 2*pr + s
    out_v = out_ap.rearrange("o (p s n) -> p s o n", p=4, s=2)

    with tile.TileContext(nc) as tc:
        with (
            tc.tile_pool(name="consts", bufs=1) as cpool,
            tc.tile_pool(name="pers", bufs=1) as pers,
            tc.tile_pool(name="smx", bufs=1) as smx,
            tc.tile_pool(name="dotsp", bufs=1, space="PSUM") as dotspool,
        ):
            wqkv_sb = cpool.tile([65, 192], F16)
            wot_sb = cpool.tile([128, 64], F32)
            ident_sb = cpool.tile([128, 128], F16)
            nc.sync.dma_start(out=wqkv_sb[:, :], in_=wqkv_d.ap()[:, :])
            nc.sync.dma_start(out=wot_sb[:, :], in_=wot_d.ap()[:, :])
            nc.sync.dma_start(out=ident_sb[:, :], in_=ident_d.ap()[:, :])

            # v in dim-partition layout: [pair, d(0:64 even head / 64:128 odd), n]
            vdp = pers.tile([128, 4 * bl], F16)
            # all four head-pair dot accumulators share one PSUM bank.
            # start=True clears has_written BANK-wide, so only the very first
            # matmul may set it; later pairs' first writes see hw=0 and
            # overwrite, which is the correct group-begin behavior.
            dots_all = dotspool.tile([128, 512], F32, name="dots")
            dots_ps = [dots_all[:, 128 * p: 128 * p + 128] for p in range(4)]

            # block-diagonal P^T per head pair; off-diag blocks stay zero
            mhbd = [smx.tile([128, 128], F16, name=f"mhbd{p}") for p in range(4)]
            for p in range(4):
                nc.vector.memset(mhbd[p][:, :], 0.0)

            # ---------------- Phase A ----------------
            vdp_v = vdp.rearrange("p (r n) -> p r n", r=4)
            with (
                tc.tile_pool(name="xq", bufs=2) as xpool,
                tc.tile_pool(name="slots", bufs=5) as slotpool,
                tc.tile_pool(name="projp", bufs=4, space="PSUM") as projpool,
                tc.tile_pool(name="vtrp", bufs=3, space="PSUM") as vtrpool,
            ):
                slots = {}

                def consume(g):
                    # dots + v-transpose for a group whose slot is fully evac'd
                    slot = slots.pop(g)
                    vt = vtrpool.tile([128, 512], F16, name="vt")
                    for pr in range(4):
                        qs = slot[:, 128 * pr: 128 * pr + 128]
                        ks = slot[:, 512 + 128 * pr: 512 + 128 * pr + 128]
                        vs = slot[:, 1024 + 128 * pr: 1024 + 128 * pr + 128]
                        nc.tensor.matmul(
                            dots_ps[pr][:, :],
                            lhsT=qs,
                            rhs=ks,
                            start=(g == 0 and pr == 0),
                            stop=(g == n_groups - 1),
                        )
                        nc.tensor.transpose(
                            vt[:, pr * 128:(pr + 1) * 128], vs, ident_sb[:, :]
                        )
                    voff = g * 128
                    vdst = vdp_v[:, :, voff:voff + 128]
                    if g % 2 == 0:
                        nc.vector.tensor_copy(vdst, vt[:, :])
                    else:
                        nc.scalar.copy(vdst, vt[:, :])

                for q in range(NQ):
                    xq = xpool.tile([65, 8 * ql], F16, name="xq")
                    # quarter-major DRAM layout: contiguous 32KB per partition
                    nc.gpsimd.dma_start(
                        out=xq[:, :],
                        in_=x_ap[:, q * 8 * ql:(q + 1) * 8 * ql],
                    )
                    for t0 in range(tpq):
                        g = q * tpq + t0
                        # slot cols: r*512 + head*64 + i*8 + alpha (alpha contiguous)
                        slot = slotpool.tile([128, 1536], F16, name="slot")
                        slot_sc = slot.rearrange(
                            "p (r h i a) -> p i r h a", r=3, h=8, i=8, a=8
                        )
                        slots[g] = slot
                        for ip in range(4):  # chunk pairs (2i, 2i+1)
                            pp = projpool.tile([128, 384], F32, name="pp")
                            for c in range(2):
                                i = 2 * ip + c
                                nc.tensor.matmul(
                                    pp[:, c * 192:(c + 1) * 192],
                                    lhsT=xq[:, i * ql + t0 * 128:
                                            i * ql + t0 * 128 + 128],
                                    rhs=wqkv_sb[:, :],
                                    start=True,
                                    stop=True,
                                )
                            dst = slot_sc[:, 2 * ip: 2 * ip + 2, :, :, :]
                            if ip % 2 == 0:
                                nc.vector.tensor_copy(dst, pp[:, :])
                            else:
                                nc.scalar.copy(dst, pp[:, :])
                        if g >= 3:
                            consume(g - 3)
                for g in (n_groups - 3, n_groups - 2, n_groups - 1):
                    consume(g)

            # ---------------- Softmax + output ----------------
            with (
                tc.tile_pool(name="mhp", bufs=2, space="PSUM") as mhpool,
                tc.tile_pool(name="finp", bufs=4, space="PSUM") as finpool,
                tc.tile_pool(name="outs", bufs=2) as outpool,
            ):
                for pr in range(4):
                    negmax = smx.tile([128, 1], F32, name=f"negmax{pr}")
                    rowsum = smx.tile([128, 1], F32, name=f"rowsum{pr}")
                    recip = smx.tile([128, 1], F32, name=f"recip{pr}")
                    exps = smx.tile([128, 64], F16, name=f"exps{pr}")
                    wots = smx.tile([128, 64], F16, name=f"wots{pr}")
                    mh_ps = mhpool.tile([128, 64], F32, name="mh_ps")
                    for s in range(2):  # head = 2*pr + s
                        b = s * 64
                        dsl = dots_ps[pr][b:b + 64, b:b + 64]
                        nc.vector.reduce_max(
                            negmax[b:b + 64, 0:1], dsl,
                            axis=mybir.AxisListType.X, negate=True,
                        )
                        nc.scalar.activation(
                            exps[b:b + 64, :], dsl,
                            mybir.ActivationFunctionType.Exp,
                            bias=negmax[b:b + 64, 0:1],
                            scale=1.0,
                            accum_out=rowsum[b:b + 64, 0:1],
                        )
                        nc.vector.reciprocal(
                            recip[b:b + 64, 0:1], rowsum[b:b + 64, 0:1]
                        )
                        nc.vector.tensor_scalar_mul(
                            wots[b:b + 64, :],
                            wot_sb[b:b + 64, :],
                            recip[b:b + 64, 0:1],
                        )
                        nc.tensor.matmul(
                            mh_ps[b:b + 64, :],
                            lhsT=exps[b:b + 64, :],
                            rhs=wots[b:b + 64, :],
                            start=True,
                            stop=True,
                        )
                        # diag block of the pair's block-diagonal P^T
                        nc.vector.tensor_copy(
                            mhbd[pr][b:b + 64, b:b + 64], mh_ps[b:b + 64, :]
                        )

                    outsb = outpool.tile([128, bl], F16, name="outsb")
                    for s5 in range(bl // 512):
                        fp_ = finpool.tile([128, 512], F32, name="fp_")
                        n0 = pr * bl + s5 * 512
                        nc.tensor.matmul(
                            fp_[:, :],
                            lhsT=mhbd[pr][:, :],
                            rhs=vdp[:, n0:n0 + 512],
                            start=True,
                            stop=True,
                        )
                        if s5 % 2 == 0:
                            nc.vector.tensor_copy(
                                outsb[:, s5 * 512:(s5 + 1) * 512], fp_[:, :])
                        else:
                            nc.scalar.copy(
                                outsb[:, s5 * 512:(s5 + 1) * 512], fp_[:, :])
                    # 4 SWDGE stores of 32 descriptors each: the packet
                    # rotation spreads consecutive calls across SDMA engines
                    for j in range(4):
                        s, o0 = j // 2, (j % 2) * 32
                        nc.gpsimd.dma_start(
                            out=out_v[pr, s, o0:o0 + 32, :],
                            in_=outsb[64 * s + o0: 64 * s + o0 + 32, :],
                        )

    nc.compile()
    return nc


_NC_CACHE = {}


def _get_nc(hw=HW):
    if hw not in _NC_CACHE:
        _NC_CACHE[hw] = _build_kernel(hw)
    return _NC_CACHE[hw]


def _host_inputs(Wq, bq, Wk, bk, Wv, bv, Wo):
    scale = 64 ** -0.5
    wqkv = np.zeros((65, 192), np.float16)
    wqkv[:64, 0:64] = (Wq.T * scale).astype(np.float16)
    wqkv[64, 0:64] = (bq * scale).astype(np.float16)
    wqkv[:64, 64:128] = Wk.T.astype(np.float16)
    wqkv[64, 64:128] = bk.astype(np.float16)
    wqkv[:64, 128:192] = Wv.T.astype(np.float16)
    wqkv[64, 128:192] = bv.astype(np.float16)
    # kernel uses c' = i*8 + alpha ordering; original c = alpha*8 + i
    pi = np.array([(c % 8) * 8 + c // 8 for c in range(64)])
    wotp = Wo.T[pi]
    wot = np.concatenate([wotp, wotp], axis=0).astype(np.float32)
    ident = np.eye(128, dtype=np.float16)
    return wqkv, wot, ident


def kernel(x, Wq, bq, Wk, bk, Wv, bv, Wo):
    global LAST_RESULTS
    B = x.shape[0]
    hw = x.shape[2] * x.shape[3]
    nc = _get_nc(hw)
    wqkv, wot, ident = _host_inputs(Wq, bq, Wk, bk, Wv, bv, Wo)

    ql = hw // HEADS // NQ
    in_maps = []
    for bidx in range(B):
        x65 = np.empty((65, hw), np.float16)
        x65[:64] = x[bidx].reshape(64, hw)
        x65[64] = 1.0
        # quarter-major column order: (q, i, ql) so each quarter load is
        # one contiguous 32KB descriptor per partition
        x65 = np.ascontiguousarray(
            x65.reshape(65, HEADS, NQ, ql).transpose(0, 2, 1, 3)
        ).reshape(65, hw)
        in_maps.append({"x": x65, "wqkv": wqkv, "wot": wot, "ident": ident})

    trace = bool(os.environ.get("KERNEL_TRACE"))
    res = run_bass_kernel_spmd(
        nc, in_maps, core_ids=list(range(B)), trace=trace
    )
    LAST_RESULTS = res
    out = np.stack(
        [res.results[bidx]["out"].reshape(64, HEADS, hw // HEADS)
         for bidx in range(B)]
    ).astype(np.float32)
    return out
